# revision 11
# baseline (speedup 1.0000x reference)
"""Trainium2 Bass kernel for CombinedRepeatCausalLinear (parallel forward).

Computes out[b,e,t] = sum_s x[b,e,s] * W[s,t] + bias[t] where
  W[s,t] = mask(t>=s) * (w0[s]*d0^(t-s) + w1[t]*d1^(t-s))
for S = 2048, x of shape (8, 1024, 2048) fp32.

Strategy (8 NeuronCores, data-parallel over batch):
  W is rank-2 + causal, so instead of the dense (S,S) matmul we use the
  chunked linear-recurrence form with chunk L=128 (16 chunks):
    out[t, r] = within-chunk triangular part + decayed cross-chunk state.
  Per core (r = 1024 rows, bf16 I/O, fp32 PSUM accumulation):
    - contraction: per chunk c one matmul with stationary Awide_c
      [128 s x 32] (zeros except cols 2c,2c+1) accumulates all chunk
      states into ONE PSUM tile P[32, r] -- the PE itself assembles the
      partition layout (engine partition access must be 32-aligned).
    - combine: one 32x32 matmul with host-precomputed chunk-decay matrix
      T32 turns P into per-chunk start-of-chunk states H[32, r]; H is
      copied into an hs tile whose row 32 is constant 1.0.
    - main: per chunk, within-chunk matmul (K=128, stationary = masked
      128x128 W block) + cross matmul (K=33, stationary Dex_c zero
      except rows 2c,2c+1 and the bias row 32) accumulate in the same
      PSUM tile; bias rides the constant-1 hs row, so PSUM->SBUF drains
      are plain copies alternating between scalar and vector engines.
  DMA instruction issue costs ~600ns each regardless of size, so
  constants are shipped as one large DMA per family (stationaries are
  free-dim slices), x as 16 full-chunk transfers, and outputs split
  across two queues.
"""

import numpy as np
import ml_dtypes

import concourse.bass as bass
import concourse.mybir as mybir
import concourse.tile as tile
from concourse import bacc
from concourse.bass_utils import run_bass_kernel_spmd

F32 = mybir.dt.float32
BF16 = mybir.dt.bfloat16
NBF16 = ml_dtypes.bfloat16

B = 8
E = 1024
S = 2048
DC = 1.0
N_CORES = 8
R = (B * E) // N_CORES      # rows per core = 1024
L = 128                     # chunk length along s/t
NCH = S // L                # 16 chunks
NST = 2 * NCH               # 32 state rows
RB = 2                      # r blocks
RBS = R // RB               # 512

_PROGRAM = None


def _build_program():
    nc = bacc.Bacc("TRN2", target_bir_lowering=False, debug=False,
                   num_devices=N_CORES)

    xT_d = nc.declare_dram_parameter("xT", [S, R], BF16, isOutput=False)
    wbig_d = nc.declare_dram_parameter("wbig", [L, NCH * L], BF16,
                                       isOutput=False)
    awbig_d = nc.declare_dram_parameter("awbig", [L, NCH * NST], BF16,
                                        isOutput=False)
    dexbig_d = nc.declare_dram_parameter("dexbig", [NST + 1, NCH * L], BF16,
                                         isOutput=False)
    t32_d = nc.declare_dram_parameter("t32", [NST, NST], BF16,
                                      isOutput=False)
    outT_d = nc.declare_dram_parameter("outT", [S, R], BF16, isOutput=True)

    with tile.TileContext(nc) as tc:
        with (
            tc.tile_pool(name="cst", bufs=1) as cst,
            tc.tile_pool(name="xp", bufs=1) as xp,
            tc.tile_pool(name="dr", bufs=4) as dr,
            tc.tile_pool(name="osb", bufs=8) as osb,
            tc.tile_pool(name="psc", bufs=2, space="PSUM") as psc,
            tc.tile_pool(name="pop", bufs=4, space="PSUM") as pop,
        ):
            # HAM primer: ~3.5us of throwaway matmuls while DMA streams in,
            # so the PE clock-gate is at K=8/8 before real work starts.
            prim = cst.tile([L, RBS], BF16, tag="prim")
            nc.gpsimd.memset(prim[:], 0.0)
            prim_ps = pop.tile([L, RBS], F32, tag="po", name="prim_ps")
            for k in range(8):
                nc.tensor.matmul(prim_ps[:], prim[:, 0:L], prim[:],
                                 start=(k == 0), stop=(k == 7))

            # resident constants, one DMA each (gpsimd queue)
            awbig = cst.tile([L, NCH * NST], BF16, tag="awbig")
            nc.gpsimd.dma_start(awbig[:], awbig_d[:])
            t32_sb = cst.tile([NST, NST], BF16, tag="t32")
            nc.gpsimd.dma_start(t32_sb[:], t32_d[:])
            wbig = cst.tile([L, NCH * L], BF16, tag="wbig")
            nc.gpsimd.dma_start(wbig[:], wbig_d[:])
            dexbig = cst.tile([NST + 1, NCH * L], BF16, tag="dexbig")
            nc.gpsimd.dma_start(dexbig[:], dexbig_d[:])

            # x tiles resident; one full-chunk DMA each, split on 2 queues
            xs = [xp.tile([L, R], BF16, tag=f"x{c}", name=f"x{c}")
                  for c in range(NCH)]
            for c in range(NCH):
                q = nc.sync if c % 2 == 0 else nc.scalar
                q.dma_start(xs[c][:], xT_d[L * c:L * (c + 1), :])

            # contractions: accumulate chunk states into P[32, r] per rb
            palls = []
            for rb in range(RB):
                palls.append(psc.tile([NST, RBS], F32, tag="pall",
                                      name=f"pall{rb}"))
            for c in range(NCH):
                aw = awbig[:, NST * c:NST * (c + 1)]
                for rb in range(RB):
                    nc.tensor.matmul(
                        palls[rb][:], aw, xs[c][:, RBS * rb:RBS * (rb + 1)],
                        start=(c == 0), stop=(c == NCH - 1))

            # combine: H = T32.T @ P; hs row 32 stays the memset 1.0
            hss = []
            for rb in range(RB):
                pall_sb = dr.tile([NST, RBS], BF16, tag="pall_sb",
                                  name=f"pallsb{rb}")
                nc.vector.tensor_copy(pall_sb[:], palls[rb][:])
                hps = psc.tile([NST, RBS], F32, tag="hps", name=f"hps{rb}")
                nc.tensor.matmul(hps[:], t32_sb[:], pall_sb[:],
                                 start=True, stop=True)
                hs = dr.tile([NST + 1, RBS], BF16, tag="hs", name=f"hs{rb}")
                nc.gpsimd.memset(hs[:], 1.0)
                nc.vector.tensor_copy(hs[0:NST, :], hps[:])
                hss.append(hs)

            # mains: within (K=128) + cross-with-bias (K=33) per (c, rb)
            for c in range(NCH):
                wl = wbig[:, L * c:L * (c + 1)]
                dx = dexbig[:, L * c:L * (c + 1)]
                for rb in range(RB):
                    po = pop.tile([L, RBS], F32, tag="po",
                                  name=f"po{rb}_{c}")
                    nc.tensor.matmul(po[:], wl,
                                     xs[c][:, RBS * rb:RBS * (rb + 1)],
                                     start=True, stop=False)
                    nc.tensor.matmul(po[:], dx, hss[rb][:],
                                     start=False, stop=True)
                    ob = osb.tile([L, RBS], BF16, tag="ob",
                                  name=f"ob{rb}_{c}")
                    if (c + rb) % 2 == 0:
                        nc.scalar.activation(
                            ob[:], po[:],
                            mybir.ActivationFunctionType.Identity)
                    else:
                        nc.vector.tensor_copy(ob[:], po[:])
                    q = nc.gpsimd if rb == 0 else nc.sync
                    q.dma_start(
                        outT_d[L * c:L * (c + 1), RBS * rb:RBS * (rb + 1)],
                        ob[:])

    nc.compile()
    return nc


def _host_prep(weight, bias, decay_value):
    w0 = weight[0].astype(np.float64)
    w1 = weight[1].astype(np.float64)
    d0 = float(np.clip(np.float32(decay_value[0, 0]), 0.9, 1.0))
    d1 = float(np.clip(np.float32(decay_value[1, 0]), 0.9, 1.0))
    ii = np.arange(L, dtype=np.float64)[:, None]   # local row (s)
    jj = np.arange(L, dtype=np.float64)[None, :]   # local col (t)
    mask = jj >= ii
    pw = np.where(mask, jj - ii, 0.0) / DC
    j1 = np.arange(L, dtype=np.float64)

    wbig = np.zeros((L, NCH * L), dtype=NBF16)
    awbig = np.zeros((L, NCH * NST), dtype=NBF16)
    dexbig = np.zeros((NST + 1, NCH * L), dtype=NBF16)
    for c in range(NCH):
        w0c = w0[L * c:L * (c + 1)]
        w1c = w1[L * c:L * (c + 1)]
        wl = np.where(mask, w0c[:, None] * d0 ** pw + w1c[None, :] * d1 ** pw,
                      0.0)
        wbig[:, L * c:L * (c + 1)] = wl.astype(NBF16)
        awbig[:, NST * c + 2 * c] = (w0c * d0 ** ((L - j1) / DC)
                                     ).astype(NBF16)
        awbig[:, NST * c + 2 * c + 1] = (d1 ** ((L - j1) / DC)
                                         ).astype(NBF16)
        dexbig[2 * c, L * c:L * (c + 1)] = (d0 ** (j1 / DC)).astype(NBF16)
        dexbig[2 * c + 1, L * c:L * (c + 1)] = (w1c * d1 ** (j1 / DC)
                                                ).astype(NBF16)
        dexbig[NST, L * c:L * (c + 1)] = bias[L * c:L * (c + 1)].astype(
            NBF16)

    t32 = np.zeros((NST, NST), dtype=NBF16)
    for c in range(NCH):          # destination chunk
        for cp in range(c):       # source chunk
            k = L * (c - cp - 1) / DC
            t32[2 * cp, 2 * c] = np.float64(d0) ** k
            t32[2 * cp + 1, 2 * c + 1] = np.float64(d1) ** k

    return wbig, awbig, dexbig, t32


def make_in_maps(x, weight, bias, decay_value):
    wbig, awbig, dexbig, t32 = _host_prep(weight, bias, decay_value)
    x2 = np.asarray(x, dtype=np.float32).reshape(B * E, S)
    in_maps = []
    for c in range(N_CORES):
        xT_c = np.ascontiguousarray(
            x2[R * c:R * (c + 1), :].T).astype(NBF16)
        in_maps.append({
            "xT": xT_c, "wbig": wbig, "awbig": awbig, "dexbig": dexbig,
            "t32": t32,
        })
    return in_maps


def kernel(x, weight, bias, decay_value, index=0, recurrent=0, **_):
    global _PROGRAM
    x = np.asarray(x, dtype=np.float32)
    weight = np.asarray(weight, dtype=np.float32)
    bias = np.asarray(bias, dtype=np.float32)
    decay_value = np.asarray(decay_value, dtype=np.float32)

    if _PROGRAM is None:
        _PROGRAM = _build_program()
    nc = _PROGRAM

    in_maps = make_in_maps(x, weight, bias, decay_value)
    res = run_bass_kernel_spmd(nc, in_maps, core_ids=list(range(N_CORES)))
    out = np.empty((B * E, S), dtype=np.float32)
    for c in range(N_CORES):
        out[R * c:R * (c + 1), :] = res.results[c]["outT"].astype(
            np.float32).T
    return out.reshape(B, E, S)
